# revision 32
# baseline (speedup 1.0000x reference)
"""Distributed multi-head attention kernel for 8 TRN2 NeuronCores.

Head-parallel tensor parallelism: each core owns 2 of the 16 heads.
Compute in bf16 (f32 PSUM accumulation). Scores are computed transposed
(ST[j,i] = k_j . q_i) so that:
  - the softmax denominator rides the PV matmul via a ones-column in V
  - no transpose of the probability matrix is needed for PV
  - the combined (bias + mask) additive tensor is pre-transposed on host
No max-subtraction softmax: logits are O(10), exp stays in f32 range.

Structure (v2): the token axis is processed in 512-token i-blocks; for
each (i-block, jt) step BOTH heads' scores live in one [128,1024] PSUM
tile (head A in cols 0:512, head B in 512:1024).  The two score matmuls
are K=64 row-tiles at PE positions (0,0)/(64,0) writing different PSUM
banks, so they can run concurrently.  One exp (ACT) and one bias-mult
(DVE) per step covers both heads.  V is transposed via the DMA xbar
(dma_start_transpose) directly into the 65-column vaug layout.
After per-head attention, bf16 head outputs are AllGathered per
512-token chunk and each core computes a 128-column slice of the output
projection, pipelined one chunk behind the gathers.
"""

import os
import numpy as np
import ml_dtypes

import concourse.bass as bass
import concourse.mybir as mybir
import concourse.tile as tile
from concourse import bacc
from concourse.bass_utils import run_bass_kernel_spmd
from concourse.masks import make_identity

BF16 = mybir.dt.bfloat16
F32 = mybir.dt.float32
AF = mybir.ActivationFunctionType
OP = mybir.AluOpType

NCORES = 8
B, N, D, H, HD = 2, 2048, 1024, 16, 64
NT = B * N            # 4096 flattened token axis, n = b*2048 + i
HPC = H // NCORES     # 2 heads per core
MASK_NEG = -30000.0
KT = D // 128         # 8 contraction tiles for the projections
NCH = NT // 512       # 8 512-token chunks / i-blocks

LAST_EXEC_TIME_NS = None


def _build_graph():
    nc = bacc.Bacc("TRN2", target_bir_lowering=False, debug=False, num_devices=NCORES)

    xT = nc.declare_dram_parameter("xT", [D, NT], BF16, isOutput=False)
    wqkvT = nc.declare_dram_parameter("wqkvT", [D, 6 * HD], BF16, isOutput=False)
    cb = nc.declare_dram_parameter("cb", [B, HPC, N, N], BF16, isOutput=False)
    wp = nc.declare_dram_parameter("wp", [D, 128], BF16, isOutput=False)
    bp = nc.declare_dram_parameter("bp", [128, 1], F32, isOutput=False)
    out_ext = nc.declare_dram_parameter("out", [128, NT], F32, isOutput=True)
    DBG = bool(os.environ.get("BASS_DEBUG_DUMP"))
    if DBG:
        dbg_qkv = nc.declare_dram_parameter("dbg_qkv", [3, 128, NT], BF16,
                                            isOutput=True)
        dbg_vaug = nc.declare_dram_parameter("dbg_vaug",
                                             [128, B * HPC * 16 * 65], BF16,
                                             isOutput=True)
        dbg_oT = nc.declare_dram_parameter("dbg_oT", [128, NT], BF16,
                                           isOutput=True)
        dbg_ot = nc.declare_dram_parameter("dbg_ot", [2, 65, 512], F32,
                                           isOutput=True)

    # collective bounce buffers, one 512-token chunk at a time
    cc_in = nc.dram_tensor("cc_in", [NCH, 128, 512], BF16)
    cc_out = nc.dram_tensor("cc_out", [NCH, NCORES * 128, 512], BF16,
                            addr_space="Shared")
    cc_warm_in = nc.dram_tensor("cc_warm_in", [1, 128], BF16)
    cc_warm_out = nc.dram_tensor("cc_warm_out", [NCORES, 128], BF16,
                                 addr_space="Shared")
    groups = [list(range(NCORES))]

    with tile.TileContext(nc) as tc:
        with (
            tc.tile_pool(name="persist", bufs=1) as persist,
            tc.tile_pool(name="st", bufs=2, space="PSUM") as st_pool,
            tc.tile_pool(name="otp", bufs=2, space="PSUM") as ot_pool,
            tc.tile_pool(name="qp", bufs=2, space="PSUM") as qp_pool,
            tc.tile_pool(name="sw", bufs=4) as sw_pool,
            tc.tile_pool(name="pw", bufs=4) as pw_pool,
            tc.tile_pool(name="cbt", bufs=6) as cb_pool,
            tc.tile_pool(name="small", bufs=4) as small_pool,
            tc.tile_pool(name="og", bufs=2) as og_pool,
            tc.tile_pool(name="outt", bufs=2) as out_pool,
            tc.tile_pool(name="otsb", bufs=4) as otsb_pool,
        ):
            # ---------------- load persistent tensors ----------------
            # tiny collective first: absorbs CC firmware init (~70us)
            # while QKV runs
            nc.sync.dma_start(out=cc_warm_in[:, :], in_=xT[0:1, 0:128])
            nc.gpsimd.collective_compute(
                "AllGather", OP.bypass, replica_groups=groups,
                ins=[cc_warm_in[:, :].opt()], outs=[cc_warm_out[:, :].opt()])

            w_sb = persist.tile([128, KT * 6 * HD], BF16, tag="w")
            for k in range(KT):
                nc.sync.dma_start(
                    out=w_sb[:, k * 6 * HD:(k + 1) * 6 * HD],
                    in_=wqkvT[k * 128:(k + 1) * 128, :])
            wp_sb = persist.tile([128, D], BF16, tag="wp")
            for k in range(KT):
                nc.sync.dma_start(out=wp_sb[:, k * 128:(k + 1) * 128],
                                  in_=wp[k * 128:(k + 1) * 128, :])
            bp_sb = persist.tile([128, 1], F32, tag="bp")
            nc.sync.dma_start(out=bp_sb[:], in_=bp[:, :])
            ones_sb = persist.tile([1, 64], BF16, tag="ones")
            nc.vector.memset(ones_sb[:], 1.0)
            id_sb = persist.tile([128, 64], BF16, tag="ident")
            make_identity(nc, id_sb[0:64, :])
            make_identity(nc, id_sb[64:128, :])
            # scratch tile: warm up the ACT exp table before attention
            warm_sb = persist.tile([1, 128], F32, tag="warm")
            nc.vector.memset(warm_sb[:], 0.0)
            nc.scalar.activation(warm_sb[:], warm_sb[:], AF.Exp)

            # x, chunked [k-tile][half] on the scalar DGE queue so QKV can
            # start after the first half-column of k-tiles; keeps the sync
            # queue free for cb prefetch
            xt_all = persist.tile([128, KT * NT], BF16, tag="xt")
            for h in range(2):
                for k in range(KT):
                    nc.scalar.dma_start(
                        out=xt_all[:, k * NT + h * 2048:
                                   k * NT + (h + 1) * 2048],
                        in_=xT[k * 128:(k + 1) * 128,
                               h * 2048:(h + 1) * 2048])

            # ---------------- QKV projection ----------------
            # qkvT_sb[m]: m=0 -> [qA;qB], m=1 -> [kA;kB], m=2 -> [vA;vB]
            qkvT_sb = [persist.tile([128, NT], BF16, tag=f"qkv{m}", name=f"qkv{m}")
                       for m in range(3)]
            q_sb, k_sb, v_sb = qkvT_sb
            # vaug: per (b, head, jt) a 65-col block [j, hd | ones]; each
            # 64-col data block is one contiguous DMA-xbar transpose, the
            # ones column comes from the initial memset.
            vaug = persist.tile([128, B * HPC * 16 * 65], BF16, tag="vaug")
            nc.vector.memset(vaug[:], 1.0)

            def emit_vt(nch):
                # PE-transpose the v chunk in [64,128] blocks into a PSUM
                # staging tile, then one DVE copy into the strided 65-col
                # vaug blocks.  (The DMA-xbar transpose path gets falsely
                # ordered behind pending collectives — avoid it.)
                b = (nch * 512) // N
                jt0 = ((nch * 512) % N) // 128
                for p in range(HPC):
                    stage = st_pool.tile([128, 4, 64], BF16, tag="st",
                                         name=f"vstg{nch}_{p}")
                    for c in range(4):
                        nc.tensor.transpose(
                            stage[:, c, :],
                            v_sb[p * 64:(p + 1) * 64,
                                 nch * 512 + c * 128:nch * 512 + (c + 1) * 128],
                            id_sb[p * 64:(p + 1) * 64, :])
                    base = ((b * HPC + p) * 16 + jt0) * 65
                    dst = vaug[:, base:base + 4 * 65]
                    dst = dst.rearrange("p (c f) -> p c f", c=4)[:, :, 0:64]
                    nc.vector.tensor_copy(dst, stage[:])

            # QKV chains emitted pairwise-interleaved so consecutive PE
            # matmuls hit alternating PSUM banks (fill/drain overlap);
            # pair outputs live in one [128,1024] st-pool tile so two
            # pairs are in flight (the PE never drains between pairs)
            def emit_qkv_pair(c0, c1):
                pt = st_pool.tile([128, 1024], F32, tag="st",
                                  name=f"qkvp{c0[0]}_{c0[1]}")
                halves = {c0: pt[:, 0:512], c1: pt[:, 512:1024]}
                for k in range(KT):
                    for (nch, m) in (c0, c1):
                        nc.tensor.matmul(
                            halves[(nch, m)],
                            lhsT=w_sb[:, k * 6 * HD + m * 128:
                                      k * 6 * HD + (m + 1) * 128],
                            rhs=xt_all[:, k * NT + nch * 512:
                                       k * NT + (nch + 1) * 512],
                            start=(k == 0), stop=(k == KT - 1))
                for (nch, m) in (c0, c1):
                    nsl = slice(nch * 512, (nch + 1) * 512)
                    nc.scalar.copy(qkvT_sb[m][:, nsl], halves[(nch, m)])
                    if m == 2:
                        emit_vt(nch)

            chains = [(nch, m) for nch in range(NCH) for m in range(3)]
            for i in range(0, len(chains), 2):
                emit_qkv_pair(chains[i], chains[i + 1])
            qkv_rest = []

            og_tiles = {}

            def emit_og(ch):
                # one 3D-AP DMA: cc_out[ch] [1024, 512] -> [128, k, 512]
                ogt = og_pool.tile([128, KT, 512], BF16, tag="og",
                                   name=f"og{ch}")
                src = cc_out[ch].rearrange("(k j) i -> j k i", j=128)
                nc.sync.dma_start(out=ogt[:], in_=src)
                og_tiles[ch] = ogt

            def emit_proj(ch):
                pps = qp_pool.tile([128, 512], F32, tag="qp",
                                   name=f"pps{ch}")
                for k in range(KT):
                    nc.tensor.matmul(pps[:],
                                     lhsT=wp_sb[:, k * 128:(k + 1) * 128],
                                     rhs=og_tiles[ch][:, k, :],
                                     start=(k == 0), stop=(k == KT - 1))
                og_tiles.pop(ch)
                outt = out_pool.tile([128, 512], F32, tag="outt",
                                     name=f"outt{ch}")
                nc.scalar.activation(outt[:], pps[:], AF.Identity,
                                     bias=bp_sb[:, 0:1])
                nc.sync.dma_start(out=out_ext[:, ch * 512:(ch + 1) * 512],
                                  in_=outt[:])

            # ---------------- attention ----------------
            # i-blocks of 512 tokens; chunk ch = global 512-token index.
            # One-step software pipeline: scores for step s+1 are emitted
            # before PV(s) so the PE FIFO never blocks the ACT exp chain.
            oT_sb = persist.tile([128, NT], BF16, tag="oT")

            def emit_cb(ch, jt):
                b = (ch * 512) // N
                jsl = slice(jt * 128, (jt + 1) * 128)
                gsl = slice((ch * 512) % N, (ch * 512) % N + 512)
                cbt = cb_pool.tile([128, 1024], BF16, tag="cbt",
                                   name=f"cbt{ch}_{jt}")
                src = cb[b, :, jsl, gsl].rearrange("p j i -> j p i")
                nc.sync.dma_start(
                    out=cbt[:].rearrange("j (p i) -> j p i", p=HPC),
                    in_=src)
                return cbt

            def emit_scores(ch, jt):
                # two K=64 row-tiled matmuls -> different PSUM banks of
                # one [128,1024] tile (concurrent on the PE array)
                b = (ch * 512) // N
                isl = slice(ch * 512, (ch + 1) * 512)
                st = st_pool.tile([128, 1024], F32, tag="st",
                                  name=f"st{ch}_{jt}")
                for p in range(HPC):
                    nc.tensor.matmul(
                        st[:, p * 512:(p + 1) * 512],
                        lhsT=k_sb[p * 64:(p + 1) * 64,
                                  b * N + jt * 128:b * N + (jt + 1) * 128],
                        rhs=q_sb[p * 64:(p + 1) * 64, isl],
                        start=True, stop=True)
                return st

            cbt_next = emit_cb(0, 0)
            st_next = emit_scores(0, 0)
            gstep = 0
            for ch in range(NCH):
                b = (ch * 512) // N
                isl = slice(ch * 512, (ch + 1) * 512)
                ots = [ot_pool.tile([65, 512], F32, tag="ot",
                                    name=f"ot{ch}_{p}")
                       for p in range(HPC)]
                for jt in range(16):
                    cbt, st = cbt_next, st_next
                    # P = exp(S) * exp(bias+mask), both heads in one pass
                    raw = sw_pool.tile([128, 1024], BF16, tag="sw",
                                       name=f"raw{ch}_{jt}")
                    nc.scalar.activation(raw[:], st[:], AF.Exp)
                    pw = pw_pool.tile([128, 1024], BF16, tag="pw",
                                      name=f"pw{ch}_{jt}")
                    nc.vector.tensor_tensor(pw[:], raw[:], cbt[:], OP.mult)
                    # prefetch next step (possibly next chunk)
                    nch_, njt = (ch, jt + 1) if jt < 15 else (ch + 1, 0)
                    if nch_ < NCH:
                        cbt_next = emit_cb(nch_, njt)
                        st_next = emit_scores(nch_, njt)
                    for p in range(HPC):
                        base = ((b * HPC + p) * 16 + jt) * 65
                        nc.tensor.matmul(
                            ots[p][:],
                            lhsT=vaug[:, base:base + 65],
                            rhs=pw[:, p * 512:(p + 1) * 512],
                            start=(jt == 0), stop=(jt == 15))
                    if qkv_rest and gstep % 5 == 2:
                        emit_qkv_pair(*qkv_rest.pop(0))
                    gstep += 1
                # Free the ot PSUM tiles quickly with two plain copies so
                # the next chunk's PV can start; normalize lazily from SBUF.
                otsb = [otsb_pool.tile([65, 512], F32, tag="otsb",
                                       name=f"otsb{ch}_{p}")
                        for p in range(HPC)]
                for p in range(HPC):
                    nc.vector.tensor_copy(otsb[p][:], ots[p][:])
                if DBG and ch == 0:
                    for p in range(HPC):
                        nc.sync.dma_start(out=dbg_ot[p], in_=otsb[p][:])
                # normalize + place into oT.  The per-token 1/sum row is
                # replicated across partitions with a K=1 PE outer product.
                for p in range(HPC):
                    sums = small_pool.tile([1, 512], F32, tag="sums",
                                           name=f"sums{ch}_{p}")
                    nc.vector.tensor_copy(sums[:], otsb[p][64:65, :])
                    recf = small_pool.tile([1, 512], F32, tag="recf",
                                           name=f"recf{ch}_{p}")
                    nc.vector.reciprocal_approx_fast(recf[:], sums[:])
                    rec = small_pool.tile([1, 512], BF16, tag="rec",
                                          name=f"rec{ch}_{p}")
                    with nc.allow_low_precision(
                            reason="bf16 softmax 1/sum"):
                        nc.vector.tensor_copy(rec[:], recf[:])
                    rep_ps = qp_pool.tile([64, 512], F32, tag="qp",
                                          name=f"rep{ch}_{p}")
                    nc.tensor.matmul(rep_ps[:], lhsT=ones_sb[:], rhs=rec[:],
                                     start=True, stop=True)
                    rep = small_pool.tile([64, 512], BF16, tag="rep",
                                          name=f"repc{ch}_{p}")
                    with nc.allow_low_precision(
                            reason="bf16 bcast of softmax 1/sum"):
                        nc.vector.tensor_copy(rep[:], rep_ps[:])
                    nc.vector.tensor_tensor(
                        oT_sb[p * 64:(p + 1) * 64, isl],
                        otsb[p][0:64, :], rep[:], OP.mult)
                nc.sync.dma_start(out=cc_in[ch], in_=oT_sb[:, isl])
                nc.gpsimd.collective_compute(
                    "AllGather", OP.bypass, replica_groups=groups,
                    ins=[cc_in[ch, :, :].opt()],
                    outs=[cc_out[ch, :, :].opt()])
                if ch >= 2:   # og+proj lag two chunks so the gather for
                    emit_og(ch - 2)    # ch-2 is already complete and the
                    emit_proj(ch - 2)  # ACT/PE FIFOs never block on CC

            emit_og(NCH - 2)
            emit_proj(NCH - 2)
            emit_og(NCH - 1)
            emit_proj(NCH - 1)
            if DBG:
                for m in range(3):
                    nc.sync.dma_start(out=dbg_qkv[m], in_=qkvT_sb[m][:])
                nc.sync.dma_start(out=dbg_vaug[:, :], in_=vaug[:])
                nc.sync.dma_start(out=dbg_oT[:, :], in_=oT_sb[:])

    nc.compile()
    return nc


_GRAPH = None


def _get_graph():
    global _GRAPH
    if _GRAPH is None:
        _GRAPH = _build_graph()
    return _GRAPH


def kernel(x, attn_bias, attn_mask, w_qkv, w_proj, b_proj):
    global LAST_EXEC_TIME_NS
    bf16 = ml_dtypes.bfloat16
    x = np.asarray(x, np.float32)
    attn_bias = np.asarray(attn_bias, np.float32)
    attn_mask = np.asarray(attn_mask)
    w_qkv = np.asarray(w_qkv, np.float32)
    w_proj = np.asarray(w_proj, np.float32)
    b_proj = np.asarray(b_proj, np.float32)

    scale = np.float32(HD ** -0.5)
    xT = np.ascontiguousarray(x.reshape(NT, D).T).astype(bf16)
    wq, wk, wv = w_qkv[0:D], w_qkv[D:2 * D], w_qkv[2 * D:3 * D]
    maskvalT = np.where(attn_mask, np.float32(MASK_NEG),
                        np.float32(0.0)).transpose(0, 2, 1)  # [B, j, i]
    biasT = attn_bias[0].transpose(0, 2, 1)                  # [H, j, i]

    in_maps = []
    for c in range(NCORES):
        hs = [HPC * c + p for p in range(HPC)]
        wcols = np.concatenate(
            [wq[h * HD:(h + 1) * HD] * scale for h in hs]
            + [wk[h * HD:(h + 1) * HD] for h in hs]
            + [wv[h * HD:(h + 1) * HD] for h in hs], axis=0)   # [384, D]
        wqkvT_np = np.ascontiguousarray(wcols.T).astype(bf16)  # [D, 384]
        cb_np = np.empty((B, HPC, N, N), dtype=bf16)
        for b in range(B):
            for p, h in enumerate(hs):
                with np.errstate(under="ignore"):
                    cb_np[b, p] = np.exp(biasT[h] + maskvalT[b]).astype(bf16)
        wp_np = np.ascontiguousarray(
            w_proj[c * 128:(c + 1) * 128, :].T).astype(bf16)   # [D, 128]
        bp_np = b_proj[c * 128:(c + 1) * 128].reshape(128, 1).astype(np.float32)
        in_maps.append({"xT": xT, "wqkvT": wqkvT_np, "cb": cb_np,
                        "wp": wp_np, "bp": bp_np})

    nc = _get_graph()
    trace = bool(os.environ.get("BASS_PROF"))
    res = run_bass_kernel_spmd(nc, in_maps, core_ids=list(range(NCORES)),
                               trace=trace)
    LAST_EXEC_TIME_NS = res.exec_time_ns
    outT = np.concatenate([res.results[i]["out"] for i in range(NCORES)],
                          axis=0)                              # [1024, NT] f32
    return np.ascontiguousarray(outT.T).reshape(B, N, D).astype(np.float32)
